# revision 1
# baseline (speedup 1.0000x reference)
"""Causal self-attention (B=4, S=2048, D=1024, single head) on 8 TRN2 cores.

Sharding: core c handles batch b = c//2 with query-tile parity p = c%2 —
its 8 query tiles of 128 rows are the absolute 128-row tiles {2j+p}.
Interleaving parities balances causal work exactly: both cores of a pair
process the same kv extent per local tile j, so the single SPMD program
is uniform; per-core variation is data-only (query rows and the additive
causal mask for the last kv group).

All matmuls run in float32r (full-rate fp32 with reduced mantissa):
  qT[o,s] = WqT.T @ xT        kT likewise       v[s,o] = xT.T @ WvT
  scores[sq,kv] = qT.T @ kT (+ identity.T @ mask on the last group)
  P = exp(scale*scores) with fused row-sum (ScalarE accum_out)
  PT = PE-transpose(P);  out[sq,o] = PT.T @ v;  out *= 1/rowsum
"""

import numpy as np

B, S, D = 4, 2048, 1024
DC = D // 128          # contraction chunks
NB = S // 128          # kv blocks per batch
NT = 8                 # q tiles per core
SCALE = 1.0 / np.sqrt(np.float32(D))
NEG = np.float32(-1e30)

_cache = {}


def _ext(j):
    # kv extent for local tile j in 128-blocks (uniform across cores);
    # rounded up to a multiple of 2 so the tail group is 256-wide
    return 2 * j + 2


def _build():
    if "nc" in _cache:
        return _cache["nc"]

    import concourse.bacc as bacc
    import concourse.mybir as mybir
    import concourse.tile as tile

    f32 = mybir.dt.float32
    f32r = mybir.dt.float32r
    AF = mybir.ActivationFunctionType

    nc = bacc.Bacc("TRN2", target_bir_lowering=False, debug=False,
                   num_devices=8)
    xq_d = nc.dram_tensor("xq", [D, NT * 128], f32r, kind="ExternalInput").ap()
    xkv_d = nc.dram_tensor("xkv", [D, S], f32r, kind="ExternalInput").ap()
    w_d = {n: nc.dram_tensor(n, [D, D], f32r, kind="ExternalInput").ap()
           for n in ("wq", "wk", "wv")}
    masks_d = nc.dram_tensor("masks", [NT * 128, 512], f32r,
                             kind="ExternalInput").ap()  # col 0..gw(j) used
    ident_d = nc.dram_tensor("ident", [128, 128], f32r,
                             kind="ExternalInput").ap()
    out_d = nc.dram_tensor("out", [NT * 128, D], f32,
                           kind="ExternalOutput").ap()

    with tile.TileContext(nc) as tc:
        with (
            tc.tile_pool(name="persist", bufs=1) as persist,
            tc.tile_pool(name="dram", bufs=1, space="DRAM") as dramp,
        ):
            kT = persist.tile([128, DC * S], f32r)          # [o%128, oc*S+kv]
            qT = persist.tile([128, DC * NT * 128], f32r)   # [o%128, oc*1024+sq]
            vtmp = [dramp.tile([512, D], f32r, name=f"vtmp{i}",
                               tag=f"vtmp{i}") for i in range(4)]

            vsb0a = persist.tile([128, 2 * D], f32r)  # kv blocks 0..1
            masks01 = persist.tile([128, 768], f32r)  # tile0 (256) + tile1 (512)
            with (
                tc.tile_pool(name="wpool", bufs=2) as wp,
                tc.tile_pool(name="xpool", bufs=2) as xs,
                tc.tile_pool(name="evpool", bufs=2) as ev,
                tc.tile_pool(name="pspj", bufs=4, space="PSUM") as psp,
            ):
                def load_w(name, cuts=(0, DC // 2, DC), eng=None):
                    w = wp.tile([128, DC * D], f32r, name=f"w_{name}",
                                tag="w")  # [d%128, dc*D + o]
                    src = w_d[name].rearrange("(c p) o -> p c o", p=128)
                    wv3 = w[:].rearrange("p (c o) -> p c o", c=DC)
                    for a, b in zip(cuts[:-1], cuts[1:]):
                        (eng or nc.sync).dma_start(wv3[:, a:b], src[:, a:b])
                    return w

                def load_x(src_ap, c0, cuts=(0, DC), eng=None):
                    xt = xs.tile([128, DC * 512], f32r, tag="x")
                    dst = xt[:].rearrange("p (c s) -> p c s", c=DC)
                    src = src_ap[:, c0 * 512:(c0 + 1) * 512] \
                        .rearrange("(c p) s -> p c s", p=128)
                    for a, b in zip(cuts[:-1], cuts[1:]):
                        (eng or nc.sync).dma_start(dst[:, a:b], src[:, a:b])
                    return xt

                warm = ev.tile([128, 1], f32, tag="warm")
                nc.gpsimd.memset(warm[:], 0.0)
                nc.scalar.activation(warm[:], warm[:], AF.Exp)

                # first-needed data first, <=1MB pieces so arrival is set
                # by dispatch rate, not a single big transfer; x and later
                # weights on the ACT HWDGE queue to parallelize dispatch
                wq = load_w("wq", cuts=(0, 1, 2, 4, 6, DC))
                xt0 = load_x(xq_d, 0, cuts=(0, 2, 4, 6, DC), eng=nc.scalar)
                xt1 = load_x(xq_d, 1, cuts=(0, 4, DC), eng=nc.scalar)
                wk = load_w("wk", eng=nc.scalar)
                nc.scalar.dma_start(masks01[:, 0:256], masks_d[0:128, 0:256])
                nc.scalar.dma_start(masks01[:, 256:768],
                                    masks_d[128:256, 0:512])

                # ---- Phase A: q projection (into resident qT) ----
                for sg in range(2):
                    xt = xt0 if sg == 0 else xt1
                    for ot in range(8):
                        ps = psp.tile([128, 512], f32, tag="pj")
                        for dc in range(DC):
                            nc.tensor.matmul(
                                ps[:],
                                wq[:, dc * D + ot * 128:dc * D + ot * 128 + 128],
                                xt[:, dc * 512:(dc + 1) * 512],
                                start=(dc == 0), stop=(dc == DC - 1))
                        nc.vector.tensor_copy(
                            qT[:, ot * 1024 + sg * 512:ot * 1024 + sg * 512 + 512],
                            ps[:])

                wv = load_w("wv")  # wq's slot; prefetches during BC

                # ---- Phase BC: k and v projections from shared x chunks ----
                for sg in range(4):
                    xt = load_x(xkv_d, sg)
                    for ot in range(8):
                        ps = psp.tile([128, 512], f32, tag="pj")
                        for dc in range(DC):
                            nc.tensor.matmul(
                                ps[:],
                                wk[:, dc * D + ot * 128:dc * D + ot * 128 + 128],
                                xt[:, dc * 512:(dc + 1) * 512],
                                start=(dc == 0), stop=(dc == DC - 1))
                        nc.vector.tensor_copy(
                            kT[:, ot * S + sg * 512:ot * S + sg * 512 + 512],
                            ps[:])
                    for st in range(4):
                        for og in range(2):
                            ps = psp.tile([128, 512], f32, tag="pj")
                            for dc in range(DC):
                                nc.tensor.matmul(
                                    ps[:],
                                    xt[:, dc * 512 + st * 128:dc * 512 + st * 128 + 128],
                                    wv[:, dc * D + og * 512:dc * D + og * 512 + 512],
                                    start=(dc == 0), stop=(dc == DC - 1))
                            vtb = ev.tile([128, 512], f32r, tag="ev")
                            nc.vector.tensor_copy(vtb[:], ps[:])
                            nc.scalar.dma_start(
                                vtmp[sg][st * 128:st * 128 + 128,
                                         og * 512:og * 512 + 512],
                                vtb[:])
                    if sg == 0:
                        nc.scalar.dma_start(
                            vsb0a[:].rearrange("p (c o) -> p c o", c=2),
                            vtmp[0][0:256, :]
                            .rearrange("(c p) o -> p c o", p=128))

            # ---- Phase D: attention ----
            with (
                tc.tile_pool(name="vD", bufs=1) as vp,
                tc.tile_pool(name="cD", bufs=1) as cp,
                tc.tile_pool(name="pD", bufs=2) as pp,
                tc.tile_pool(name="ptD", bufs=1) as ptp,
                tc.tile_pool(name="oD", bufs=1) as op,
                tc.tile_pool(name="smD", bufs=2) as smp,
                tc.tile_pool(name="psS", bufs=2, space="PSUM") as ps_s,
                tc.tile_pool(name="psT", bufs=2, space="PSUM") as ps_t,
                tc.tile_pool(name="psO", bufs=2, space="PSUM") as ps_o,
            ):
                ident = cp.tile([128, 128], f32r)
                nc.sync.dma_start(ident[:], ident_d)
                # vsb[0] holds only blocks 2..3 (0..1 live in vsb0a)
                vsb = [vp.tile([128, (2 if q4 == 0 else 4) * D], f32r,
                               name=f"vsb{q4}", tag=f"v{q4}")
                       for q4 in range(4)]

                def load_v(q4):
                    nb = 2 if q4 == 0 else 4
                    src_rows = vtmp[q4][512 - nb * 128:512, :] \
                        .rearrange("(c p) o -> p c o", p=128)
                    dst = vsb[q4][:].rearrange("p (c o) -> p c o", c=nb)
                    for c in range(0, nb, 2):
                        nc.sync.dma_start(dst[:, c:c + 2],
                                          src_rows[:, c:c + 2])

                load_v(0)
                masks = cp.tile([128, NT * 512], f32r)  # [p, j*512+kv]
                nc.sync.dma_start(
                    masks[:].rearrange("p (j k) -> p j k", j=NT),
                    masks_d.rearrange("(j p) k -> p j k", p=128))
                for q4 in range(1, 4):
                    load_v(q4)

                def vblk(kb):
                    if kb < 2:
                        return vsb0a[:, kb * D:(kb + 1) * D]
                    if kb < 4:
                        return vsb[0][:, (kb - 2) * D:(kb - 1) * D]
                    return vsb[kb // 4][:, (kb % 4) * D:(kb % 4 + 1) * D]

                Ph, dsh, rch = {}, {}, {}

                def scores_exp(j):
                    ext = _ext(j)
                    ng = (ext + 3) // 4
                    P = pp.tile([128, NB * 128], f32r, tag="P",
                                name=f"P{j}")
                    dslots = smp.tile([128, 4], f32, tag="ds",
                                      name=f"ds{j}")
                    for g in range(ng):
                        gw = min(512, ext * 128 - g * 512)
                        last = (g == ng - 1)
                        sps = ps_s.tile([128, 512], f32, tag="sc",
                                        name=f"sps{j}_{g}")
                        for oc in range(DC):
                            nc.tensor.matmul(
                                sps[:, 0:gw],
                                qT[:, oc * 1024 + j * 128:oc * 1024 + j * 128 + 128],
                                kT[:, oc * S + g * 512:oc * S + g * 512 + gw],
                                start=(oc == 0),
                                stop=(oc == DC - 1 and not last))
                        if last:
                            msrc = (masks01[:, 256 * j:256 * j + gw]
                                    if j < 2 else
                                    masks[:, j * 512:j * 512 + gw])
                            nc.tensor.matmul(sps[:, 0:gw], ident[:], msrc,
                                             start=False, stop=True)
                        nc.scalar.activation(
                            P[:, g * 512:g * 512 + gw], sps[:, 0:gw], AF.Exp,
                            scale=float(SCALE),
                            accum_out=dslots[:, g:g + 1])
                    rcp = smp.tile([128, 1], f32, tag="rcp", name=f"rcp{j}")
                    den = smp.tile([128, 1], f32, tag="den", name=f"den{j}")
                    nc.vector.reduce_sum(den[:], dslots[:, 0:ng],
                                         axis=mybir.AxisListType.X)
                    nc.vector.reciprocal(rcp[:], den[:])
                    Ph[j], dsh[j], rch[j] = P, dslots, rcp

                def transp_av(j):
                    ext = _ext(j)
                    ng = (ext + 3) // 4
                    P, rcp = Ph[j], rch[j]
                    PT = ptp.tile([128, NB * 128], f32r, tag="PT",
                                  name=f"PT{j}")
                    for g in range(ng):
                        nb = min(4, ext - g * 4)
                        tps = ps_t.tile([128, 512], f32r, tag="tp",
                                        name=f"tps{j}_{g}")
                        for bb in range(nb):
                            nc.tensor.transpose(
                                tps[:, bb * 128:(bb + 1) * 128],
                                P[:, g * 512 + bb * 128:g * 512 + bb * 128 + 128],
                                ident[:])
                        nc.vector.tensor_copy(
                            PT[:, g * 512:g * 512 + nb * 128],
                            tps[:, 0:nb * 128])

                    ops = ps_o.tile([128, D], f32, tag="av", name=f"ops{j}")
                    for og in range(2):
                        for kb in range(ext):
                            nc.tensor.matmul(
                                ops[:, og * 512:(og + 1) * 512],
                                PT[:, kb * 128:(kb + 1) * 128],
                                vblk(kb)[:, og * 512:(og + 1) * 512],
                                start=(kb == 0), stop=(kb == ext - 1))
                    osb = op.tile([128, D], f32, tag="o", name=f"o{j}")
                    nc.vector.tensor_scalar_mul(osb[:], ops[:], rcp[:])
                    nc.sync.dma_start(out_d[j * 128:(j + 1) * 128, :], osb[:])

                scores_exp(0)
                for j in range(NT):
                    if j + 1 < NT:
                        scores_exp(j + 1)
                    transp_av(j)

    nc.compile()
    _cache["nc"] = nc
    return nc


def _shard(x, Wq, Wk, Wv):
    """Build the 8 per-core input maps from full inputs."""
    ident = np.eye(128, dtype=np.float32)
    wqt = np.ascontiguousarray(Wq.T)
    wkt = np.ascontiguousarray(Wk.T)
    wvt = np.ascontiguousarray(Wv.T)
    in_maps = []
    for c in range(8):
        b, p = c // 2, c % 2
        xb = x[b]
        xkv = np.ascontiguousarray(xb.T)
        rows = np.concatenate(
            [xb[(2 * j + p) * 128:(2 * j + p + 1) * 128, :] for j in range(8)],
            axis=0)
        xq = np.ascontiguousarray(rows.T)
        masks = np.full((NT * 128, 512), NEG, np.float32)
        for j in range(NT):
            ext = _ext(j)
            ng = (ext + 3) // 4
            gw = min(512, ext * 128 - (ng - 1) * 512)
            q_abs = (2 * j + p) * 128 + np.arange(128)[:, None]
            kv_abs = (ng - 1) * 512 + np.arange(gw)[None, :]
            masks[j * 128:(j + 1) * 128, 0:gw] = np.where(
                kv_abs <= q_abs, np.float32(0), NEG)
        in_maps.append({
            "xq": xq, "xkv": xkv, "wq": wqt, "wk": wkt, "wv": wvt,
            "masks": masks, "ident": ident,
        })
    return in_maps


def _unshard(results, dtype):
    out = np.empty((B, S, D), dtype)
    for c in range(8):
        b, p = c // 2, c % 2
        o = results[c]["out"]
        for j in range(NT):
            out[b, (2 * j + p) * 128:(2 * j + p + 1) * 128, :] = \
                o[j * 128:(j + 1) * 128, :]
    return out


def run(x, Wq, Wk, Wv, trace=False):
    from concourse.bass_utils import run_bass_kernel_spmd
    nc = _build()
    in_maps = _shard(np.asarray(x), np.asarray(Wq), np.asarray(Wk),
                     np.asarray(Wv))
    res = run_bass_kernel_spmd(nc, in_maps, core_ids=list(range(8)),
                               trace=trace)
    return _unshard(res.results, np.float32), res


def kernel(x, Wq, Wk, Wv):
    out, _ = run(x, Wq, Wk, Wv, trace=False)
    return out



# revision 2
# speedup vs baseline: 1.3164x; 1.3164x over previous
"""Causal self-attention (B=4, S=2048, D=1024, single head) on 8 TRN2 cores.

Sharding: core c handles batch b = c//2 with query-tile parity p = c%2 —
its 8 query tiles of 128 rows are the absolute 128-row tiles {2j+p}.
Interleaving parities balances causal work exactly; the single SPMD
program is uniform and per-core variation is data-only (query rows and
the additive causal mask for the last kv group).

Re-associated algebra removes the k and v projections entirely:
  scores = q @ k.T = ((x_q Wq^T) Wk) @ x^T      (only q-rows projected)
  out    = attn @ v = (attn @ x) @ Wv^T          (project the context)
Per-core tensor work drops from ~15.9 GF to ~11.8 GF with zero
cross-core duplication and no DRAM round-trip for v.

All matmul operands are bf16 (fp32 PSUM accumulation): full PE rate at
any free-dim width, 1.0 cyc/row transposes, and half the DMA bytes.

Phases (PE program order):
  A: qT[o,sq]   = wqT.T @ xqT          B: qkT[d,sq] = wk.T @ qT
  per tile j:  scores[sq,kv] = qkT.T @ xkvT (+ ident.T @ mask, last grp)
               P = exp(scale*scores), fused row-sum (ScalarE accum_out)
               PT = PE-transpose(P);   ctx[sq,d] = PT.T @ xnat
               ctx *= 1/rowsum (cast bf16);  ctxT = PE-transpose(ctx)
               out[sq,o] = ctxT.T @ wvT
"""

import numpy as np
import ml_dtypes

B, S, D = 4, 2048, 1024
DC = D // 128          # 128-chunks along d / o
NB = S // 128          # kv blocks per batch
NT = 8                 # q tiles per core
SCALE = 1.0 / np.sqrt(np.float32(D))
NEG = np.float32(-1e30)
BF16 = ml_dtypes.bfloat16

_cache = {}


def _ext(j):
    # kv extent for local tile j in 128-blocks (uniform across cores);
    # rounded up to a multiple of 2 so the tail group is 256-wide
    return 2 * j + 2


def _build():
    if "nc" in _cache:
        return _cache["nc"]

    import concourse.bacc as bacc
    import concourse.mybir as mybir
    import concourse.tile as tile

    f32 = mybir.dt.float32
    bf16 = mybir.dt.bfloat16
    AF = mybir.ActivationFunctionType

    nc = bacc.Bacc("TRN2", target_bir_lowering=False, debug=False,
                   num_devices=8)
    xq_d = nc.dram_tensor("xq", [D, NT * 128], bf16, kind="ExternalInput").ap()
    xkv_d = nc.dram_tensor("xkv", [D, S], bf16, kind="ExternalInput").ap()
    xn_d = nc.dram_tensor("xn", [S, D], bf16, kind="ExternalInput").ap()
    w_d = {n: nc.dram_tensor(n, [D, D], bf16, kind="ExternalInput").ap()
           for n in ("wq", "wk", "wv")}
    masks_d = nc.dram_tensor("masks", [NT * 128, 512], bf16,
                             kind="ExternalInput").ap()  # col 0..gw(j) used
    ident_d = nc.dram_tensor("ident", [128, 128], bf16,
                             kind="ExternalInput").ap()
    out_d = nc.dram_tensor("out", [NT * 128, D], f32,
                           kind="ExternalOutput").ap()

    with tile.TileContext(nc) as tc:
        with (
            tc.tile_pool(name="persist", bufs=1) as persist,
            tc.tile_pool(name="wpool", bufs=2) as wp,
        ):
            xkvT = persist.tile([128, DC * S], bf16)        # [d%128, dc*S+kv]
            xnat = persist.tile([128, NB * D], bf16)        # [s%128, kb*D+d]
            qT = persist.tile([128, DC * NT * 128], bf16)   # [o%128, oc*1k+sq]
            qkT = persist.tile([128, DC * NT * 128], bf16)  # [d%128, dc*1k+sq]
            masks = persist.tile([128, NT * 512], bf16)     # [p, j*512+kv]
            ident = persist.tile([128, 128], bf16)
            warm = persist.tile([128, 1], f32)

            def load_w(name, cuts, eng):
                w = wp.tile([128, DC * D], bf16, name=f"w_{name}", tag="w")
                src = w_d[name].rearrange("(c p) o -> p c o", p=128)
                wv3 = w[:].rearrange("p (c o) -> p c o", c=DC)
                for a, b in zip(cuts[:-1], cuts[1:]):
                    eng.dma_start(wv3[:, a:b], src[:, a:b])
                return w

            with (
                tc.tile_pool(name="xqpool", bufs=1) as xqs,
                tc.tile_pool(name="psAB", bufs=4, space="PSUM") as psA,
            ):
                # warm-up: trigger ACT_TABLE_LOAD for Exp early
                nc.gpsimd.memset(warm[:], 0.0)
                nc.scalar.activation(warm[:], warm[:], AF.Exp)

                # ---- input DMA, first-needed first, 3 parallel queues ----
                wq = load_w("wq", cuts=(0, 1, 2, 4, 6, DC), eng=nc.sync)
                xq_sb = xqs.tile([128, DC * NT * 128], bf16, tag="xq")
                xq3 = xq_sb[:].rearrange("p (c s) -> p c s", c=DC)
                xq_src = xq_d.rearrange("(c p) s -> p c s", p=128)
                for a, b in zip((0, 1, 2, 4, 6, DC)[:-1], (0, 1, 2, 4, 6, DC)[1:]):
                    nc.scalar.dma_start(xq3[:, a:b], xq_src[:, a:b])
                nc.gpsimd.dma_start(ident[:], ident_d)
                wk = load_w("wk", cuts=(0, 4, DC), eng=nc.sync)
                nc.gpsimd.dma_start(
                    masks[:].rearrange("p (j k) -> p j k", j=NT),
                    masks_d.rearrange("(j p) k -> p j k", p=128))
                xkv3 = xkvT[:].rearrange("p (c s) -> p c s", c=DC)
                for wnd in range(4):
                    src = xkv_d[:, wnd * 512:(wnd + 1) * 512] \
                        .rearrange("(c p) s -> p c s", p=128)
                    nc.scalar.dma_start(xkv3[:, :, wnd * 512:(wnd + 1) * 512],
                                        src)
                wv = load_w("wv", cuts=(0, 4, DC), eng=nc.sync)
                xn3 = xnat[:].rearrange("p (c s) -> p c s", c=NB)
                xn_src = xn_d.rearrange("(c p) s -> p c s", p=128)
                for a in range(0, NB, 4):
                    nc.gpsimd.dma_start(xn3[:, a:a + 4], xn_src[:, a:a + 4])

                # ---- Phase A: q projection (into resident qT) ----
                for sg in range(2):
                    for ot in range(8):
                        ps = psA.tile([128, 512], f32, tag="pj")
                        for dc in range(DC):
                            nc.tensor.matmul(
                                ps[:],
                                wq[:, dc * D + ot * 128:dc * D + ot * 128 + 128],
                                xq_sb[:, dc * 1024 + sg * 512:dc * 1024 + sg * 512 + 512],
                                start=(dc == 0), stop=(dc == DC - 1))
                        nc.vector.tensor_copy(
                            qT[:, ot * 1024 + sg * 512:ot * 1024 + sg * 512 + 512],
                            ps[:])

                # ---- Phase B: fold Wk into q (qk = q @ Wk, transposed) ----
                for sg in range(2):
                    for dc in range(DC):
                        ps = psA.tile([128, 512], f32, tag="pj")
                        for oc in range(DC):
                            nc.tensor.matmul(
                                ps[:],
                                wk[:, oc * D + dc * 128:oc * D + dc * 128 + 128],
                                qT[:, oc * 1024 + sg * 512:oc * 1024 + sg * 512 + 512],
                                start=(oc == 0), stop=(oc == DC - 1))
                        nc.scalar.copy(
                            qkT[:, dc * 1024 + sg * 512:dc * 1024 + sg * 512 + 512],
                            ps[:])

            # ---- attention ----
            with (
                tc.tile_pool(name="pD", bufs=2) as pp,
                tc.tile_pool(name="ptD", bufs=1) as ptp,
                tc.tile_pool(name="cD", bufs=2) as cp,
                tc.tile_pool(name="ctD", bufs=2) as ctp,
                tc.tile_pool(name="oD", bufs=2) as op,
                tc.tile_pool(name="smD", bufs=2) as smp,
                tc.tile_pool(name="psS", bufs=2, space="PSUM") as ps_s,
                tc.tile_pool(name="psT", bufs=2, space="PSUM") as ps_t,
                tc.tile_pool(name="psM", bufs=4, space="PSUM") as ps_m,
            ):
                Ph, rch = {}, {}

                def scores_exp(j):
                    ext = _ext(j)
                    ng = (ext + 3) // 4
                    P = pp.tile([128, NB * 128], bf16, tag="P", name=f"P{j}")
                    dslots = smp.tile([128, 4], f32, tag="ds", name=f"ds{j}")
                    for g in range(ng):
                        gw = min(512, ext * 128 - g * 512)
                        last = (g == ng - 1)
                        sps = ps_s.tile([128, 512], f32, tag="sc",
                                        name=f"sps{j}_{g}")
                        for dc in range(DC):
                            nc.tensor.matmul(
                                sps[:, 0:gw],
                                qkT[:, dc * 1024 + j * 128:dc * 1024 + j * 128 + 128],
                                xkvT[:, dc * S + g * 512:dc * S + g * 512 + gw],
                                start=(dc == 0),
                                stop=(dc == DC - 1 and not last))
                        if last:
                            nc.tensor.matmul(
                                sps[:, 0:gw], ident[:],
                                masks[:, j * 512:j * 512 + gw],
                                start=False, stop=True)
                        nc.scalar.activation(
                            P[:, g * 512:g * 512 + gw], sps[:, 0:gw], AF.Exp,
                            scale=float(SCALE),
                            accum_out=dslots[:, g:g + 1])
                    rcp = smp.tile([128, 1], f32, tag="rcp", name=f"rcp{j}")
                    den = smp.tile([128, 1], f32, tag="den", name=f"den{j}")
                    nc.vector.reduce_sum(den[:], dslots[:, 0:ng],
                                         axis=mybir.AxisListType.X)
                    nc.vector.reciprocal(rcp[:], den[:])
                    Ph[j], rch[j] = P, rcp

                def tail(j):
                    ext = _ext(j)
                    ng = (ext + 3) // 4
                    P, rcp = Ph[j], rch[j]
                    PT = ptp.tile([128, NB * 128], bf16, tag="PT",
                                  name=f"PT{j}")
                    for g in range(ng):
                        nb = min(4, ext - g * 4)
                        tps = ps_t.tile([128, 512], bf16, tag="tp",
                                        name=f"tps{j}_{g}")
                        for bb in range(nb):
                            nc.tensor.transpose(
                                tps[:, bb * 128:(bb + 1) * 128],
                                P[:, g * 512 + bb * 128:g * 512 + bb * 128 + 128],
                                ident[:])
                        nc.vector.tensor_copy(
                            PT[:, g * 512:g * 512 + nb * 128],
                            tps[:, 0:nb * 128])

                    ctx = cp.tile([128, D], bf16, tag="ctx", name=f"ctx{j}")
                    for og in range(2):
                        ops = ps_m.tile([128, 512], f32, tag="av",
                                        name=f"av{j}_{og}")
                        for kb in range(ext):
                            nc.tensor.matmul(
                                ops[:],
                                PT[:, kb * 128:(kb + 1) * 128],
                                xnat[:, kb * D + og * 512:kb * D + og * 512 + 512],
                                start=(kb == 0), stop=(kb == ext - 1))
                        nc.vector.tensor_scalar_mul(
                            ctx[:, og * 512:(og + 1) * 512], ops[:], rcp[:])

                    ctxT = ctp.tile([128, D], bf16, tag="ctxT",
                                    name=f"ctxT{j}")
                    for h in range(2):
                        tps = ps_t.tile([128, 512], bf16, tag="tp",
                                        name=f"tpc{j}_{h}")
                        for q4 in range(4):
                            dc = h * 4 + q4
                            nc.tensor.transpose(
                                tps[:, q4 * 128:(q4 + 1) * 128],
                                ctx[:, dc * 128:dc * 128 + 128],
                                ident[:])
                        nc.vector.tensor_copy(
                            ctxT[:, h * 512:(h + 1) * 512], tps[:])

                    osb = op.tile([128, D], f32, tag="o", name=f"o{j}")
                    for og in range(2):
                        ops = ps_m.tile([128, 512], f32, tag="av",
                                        name=f"op{j}_{og}")
                        for dc in range(DC):
                            nc.tensor.matmul(
                                ops[:],
                                ctxT[:, dc * 128:dc * 128 + 128],
                                wv[:, dc * D + og * 512:dc * D + og * 512 + 512],
                                start=(dc == 0), stop=(dc == DC - 1))
                        nc.vector.tensor_copy(
                            osb[:, og * 512:(og + 1) * 512], ops[:])
                    nc.sync.dma_start(out_d[j * 128:(j + 1) * 128, :], osb[:])

                scores_exp(0)
                for j in range(NT):
                    if j + 1 < NT:
                        scores_exp(j + 1)
                    tail(j)

    nc.compile()
    _cache["nc"] = nc
    return nc


def _shard(x, Wq, Wk, Wv):
    """Build the 8 per-core input maps from full inputs."""
    ident = np.eye(128, dtype=np.float32).astype(BF16)
    wqt = np.ascontiguousarray(Wq.T).astype(BF16)
    wkn = np.ascontiguousarray(Wk).astype(BF16)
    wvt = np.ascontiguousarray(Wv.T).astype(BF16)
    in_maps = []
    for c in range(8):
        b, p = c // 2, c % 2
        xb = x[b]
        xkv = np.ascontiguousarray(xb.T).astype(BF16)
        xn = np.ascontiguousarray(xb).astype(BF16)
        rows = np.concatenate(
            [xb[(2 * j + p) * 128:(2 * j + p + 1) * 128, :] for j in range(8)],
            axis=0)
        xq = np.ascontiguousarray(rows.T).astype(BF16)
        masks = np.full((NT * 128, 512), NEG, np.float32)
        for j in range(NT):
            ext = _ext(j)
            ng = (ext + 3) // 4
            gw = min(512, ext * 128 - (ng - 1) * 512)
            q_abs = (2 * j + p) * 128 + np.arange(128)[:, None]
            kv_abs = (ng - 1) * 512 + np.arange(gw)[None, :]
            masks[j * 128:(j + 1) * 128, 0:gw] = np.where(
                kv_abs <= q_abs, np.float32(0), NEG)
        in_maps.append({
            "xq": xq, "xkv": xkv, "xn": xn,
            "wq": wqt, "wk": wkn, "wv": wvt,
            "masks": masks.astype(BF16), "ident": ident,
        })
    return in_maps


def _unshard(results, dtype):
    out = np.empty((B, S, D), dtype)
    for c in range(8):
        b, p = c // 2, c % 2
        o = results[c]["out"]
        for j in range(NT):
            out[b, (2 * j + p) * 128:(2 * j + p + 1) * 128, :] = \
                o[j * 128:(j + 1) * 128, :]
    return out


def run(x, Wq, Wk, Wv, trace=False):
    from concourse.bass_utils import run_bass_kernel_spmd
    nc = _build()
    in_maps = _shard(np.asarray(x), np.asarray(Wq), np.asarray(Wk),
                     np.asarray(Wv))
    res = run_bass_kernel_spmd(nc, in_maps, core_ids=list(range(8)),
                               trace=trace)
    return _unshard(res.results, np.float32), res


def kernel(x, Wq, Wk, Wv):
    out, _ = run(x, Wq, Wk, Wv, trace=False)
    return out


# revision 3
# speedup vs baseline: 1.4526x; 1.1035x over previous
"""Causal self-attention (B=4, S=2048, D=1024, single head) on 8 TRN2 cores.

Sharding: core c handles batch b = c//2 with query-tile parity p = c%2 —
its 8 query tiles of 128 rows are the absolute 128-row tiles {2j+p}.
Interleaving parities balances causal work exactly; the single SPMD
program is uniform and per-core variation is data-only (query rows and
the additive causal mask for the last kv group).

Re-associated algebra removes the k and v projections entirely:
  scores = q @ k.T = ((x_q Wq^T) Wk) @ x^T      (only q-rows projected)
  out    = attn @ v = (attn @ x) @ Wv^T          (project the context)

All matmul operands are bf16 (fp32 PSUM accumulation). Every DRAM input
is pre-arranged host-side into its exact SBUF flat layout, so each DMA
is a contiguous [128, cols] block copy ordered to match consumption.
The PE is pre-warmed with dummy matmuls during the initial DMA wait
(post-idle it runs at reduced p-state for ~3us).

PE program order (software-pipelined):
  A: qT[o,sq] = wqT.T @ xqT          B: qkT[d,sq] = wk.T @ qT
  S(0) Ptr(0) ctx(0) | S(1) | for j>=1: ctxT(j-1) Ptr(j) out(j-1)
  ctx(j) S(j+1) | ctxT(7) out(7)
where S = scores+exp (fused row-sum), Ptr = PE-transpose of P,
ctx = PT.T @ xnat (then *1/rowsum, cast bf16), ctxT = PE-transpose,
out = ctxT.T @ wvT.
"""

import numpy as np
import ml_dtypes

B, S, D = 4, 2048, 1024
DC = D // 128          # 128-chunks along d / o
NB = S // 128          # kv blocks per batch
NT = 8                 # q tiles per core
SCALE = 1.0 / np.sqrt(np.float32(D))
NEG = np.float32(-1e30)
BF16 = ml_dtypes.bfloat16

_cache = {}


def _ext(j):
    # kv extent for local tile j in 128-blocks (uniform across cores);
    # rounded up to a multiple of 2 so the tail group is 256-wide
    return 2 * j + 2


def _build():
    if "nc" in _cache:
        return _cache["nc"]

    import concourse.bacc as bacc
    import concourse.mybir as mybir
    import concourse.tile as tile

    f32 = mybir.dt.float32
    bf16 = mybir.dt.bfloat16
    AF = mybir.ActivationFunctionType

    nc = bacc.Bacc("TRN2", target_bir_lowering=False, debug=False,
                   num_devices=8)
    # all inputs pre-arranged host-side to the SBUF layout, [128, cols]
    xq_d = nc.dram_tensor("xq", [128, 2 * DC * 512], bf16,
                          kind="ExternalInput").ap()    # [p, sg,dc,s]
    xkv_d = nc.dram_tensor("xkv", [128, DC * S], bf16,
                           kind="ExternalInput").ap()   # [p, dc,kv]
    xn_d = nc.dram_tensor("xn", [128, NB * D], bf16,
                          kind="ExternalInput").ap()    # [p, kb,d]
    wq_d = nc.dram_tensor("wq", [128, DC * D], bf16,
                          kind="ExternalInput").ap()    # [p, ot,dc,oo]
    wk_d = nc.dram_tensor("wk", [128, DC * D], bf16,
                          kind="ExternalInput").ap()    # [p, dc,oc,dd]
    wv_d = nc.dram_tensor("wv", [128, DC * D], bf16,
                          kind="ExternalInput").ap()    # [p, dc,o]
    masks_d = nc.dram_tensor("masks", [128, NT * 512], bf16,
                             kind="ExternalInput").ap()  # [p, j,k]
    ident_d = nc.dram_tensor("ident", [128, 128], bf16,
                             kind="ExternalInput").ap()
    out_d = nc.dram_tensor("out", [NT * 128, D], f32,
                           kind="ExternalOutput").ap()

    with tile.TileContext(nc) as tc:
        with (
            tc.tile_pool(name="persist", bufs=1) as persist,
            tc.tile_pool(name="wpool", bufs=2) as wp,
        ):
            xkvT = persist.tile([128, DC * S], bf16)
            xnat = persist.tile([128, NB * D], bf16)
            qT = persist.tile([128, DC * NT * 128], bf16)
            qkT = persist.tile([128, DC * NT * 128], bf16)
            masks = persist.tile([128, NT * 512], bf16)
            ident = persist.tile([128, 128], bf16)
            warm = persist.tile([128, 1], f32)
            scr = persist.tile([128, 512], bf16)

            def load(dst, src, cuts, eng):
                for a, b in zip(cuts[:-1], cuts[1:]):
                    eng.dma_start(dst[:, a:b], src[:, a:b])

            with (
                tc.tile_pool(name="xqpool", bufs=1) as xqs,
                tc.tile_pool(name="psAB", bufs=4, space="PSUM") as psA,
            ):
                # warm-up: Exp table load + PE p-state ramp during DMA wait
                nc.gpsimd.memset(warm[:], 0.0)
                nc.scalar.activation(warm[:], warm[:], AF.Exp)
                nc.gpsimd.memset(scr[:], 0.0)

                # ---- input DMA: contiguous pieces, first-needed first ----
                xq_sb = xqs.tile([128, 2 * DC * 512], bf16, tag="xq")
                wq = wp.tile([128, DC * D], bf16, name="w_wq", tag="w")
                wk = wp.tile([128, DC * D], bf16, name="w_wk", tag="w")
                K = 1024
                load(wq[:], wq_d, (0, 2 * K, 4 * K, 6 * K, 8 * K), nc.sync)
                load(xq_sb[:], xq_d, (0, 2 * K, 4 * K, 6 * K, 8 * K),
                     nc.scalar)
                nc.scalar.dma_start(ident[:], ident_d)
                load(wk[:], wk_d, (0, 4 * K, 8 * K), nc.sync)
                nc.scalar.dma_start(masks[:], masks_d)
                load(xkvT[:], xkv_d, (0, 4 * K, 8 * K, 12 * K, 16 * K),
                     nc.sync)
                wv = wp.tile([128, DC * D], bf16, name="w_wv", tag="w")
                load(wv[:], wv_d, (0, 4 * K, 8 * K), nc.sync)
                load(xnat[:], xn_d, (0, 4 * K, 8 * K, 12 * K, 16 * K),
                     nc.scalar)

                # PE p-state warm-up: dummy matmuls on memset scratch
                for _ in range(8):
                    ps = psA.tile([128, 512], f32, tag="pj")
                    nc.tensor.matmul(ps[:], scr[:, 0:128], scr[:],
                                     start=True, stop=True)

                # ---- Phase A: q projection (into resident qT) ----
                for sg in range(2):
                    for ot in range(8):
                        ps = psA.tile([128, 512], f32, tag="pj")
                        for dc in range(DC):
                            nc.tensor.matmul(
                                ps[:],
                                wq[:, ot * 1024 + dc * 128:ot * 1024 + dc * 128 + 128],
                                xq_sb[:, sg * 4096 + dc * 512:sg * 4096 + dc * 512 + 512],
                                start=(dc == 0), stop=(dc == DC - 1))
                        nc.vector.tensor_copy(
                            qT[:, ot * 1024 + sg * 512:ot * 1024 + sg * 512 + 512],
                            ps[:])

                # ---- Phase B: fold Wk into q (qk = q @ Wk, transposed) ----
                for sg in range(2):
                    for dc in range(DC):
                        ps = psA.tile([128, 512], f32, tag="pj")
                        for oc in range(DC):
                            nc.tensor.matmul(
                                ps[:],
                                wk[:, dc * 1024 + oc * 128:dc * 1024 + oc * 128 + 128],
                                qT[:, oc * 1024 + sg * 512:oc * 1024 + sg * 512 + 512],
                                start=(oc == 0), stop=(oc == DC - 1))
                        nc.scalar.copy(
                            qkT[:, dc * 1024 + sg * 512:dc * 1024 + sg * 512 + 512],
                            ps[:])

            # ---- attention ----
            with (
                tc.tile_pool(name="pD", bufs=2) as pp,
                tc.tile_pool(name="ptD", bufs=1) as ptp,
                tc.tile_pool(name="cD", bufs=2) as cp,
                tc.tile_pool(name="ctD", bufs=2) as ctp,
                tc.tile_pool(name="oD", bufs=2) as op,
                tc.tile_pool(name="smD", bufs=2) as smp,
                tc.tile_pool(name="psS", bufs=2, space="PSUM") as ps_s,
                tc.tile_pool(name="psT", bufs=2, space="PSUM") as ps_t,
                tc.tile_pool(name="psM", bufs=4, space="PSUM") as ps_m,
            ):
                Ph, rch, cth = {}, {}, {}

                def scores_exp(j):
                    ext = _ext(j)
                    ng = (ext + 3) // 4
                    P = pp.tile([128, NB * 128], bf16, tag="P", name=f"P{j}")
                    dslots = smp.tile([128, 4], f32, tag="ds", name=f"ds{j}")
                    for g in range(ng):
                        gw = min(512, ext * 128 - g * 512)
                        last = (g == ng - 1)
                        sps = ps_s.tile([128, 512], f32, tag="sc",
                                        name=f"sps{j}_{g}")
                        for dc in range(DC):
                            nc.tensor.matmul(
                                sps[:, 0:gw],
                                qkT[:, dc * 1024 + j * 128:dc * 1024 + j * 128 + 128],
                                xkvT[:, dc * S + g * 512:dc * S + g * 512 + gw],
                                start=(dc == 0),
                                stop=(dc == DC - 1 and not last))
                        if last:
                            nc.tensor.matmul(
                                sps[:, 0:gw], ident[:],
                                masks[:, j * 512:j * 512 + gw],
                                start=False, stop=True)
                        nc.scalar.activation(
                            P[:, g * 512:g * 512 + gw], sps[:, 0:gw], AF.Exp,
                            scale=float(SCALE),
                            accum_out=dslots[:, g:g + 1])
                    rcp = smp.tile([128, 1], f32, tag="rcp", name=f"rcp{j}")
                    den = smp.tile([128, 1], f32, tag="den", name=f"den{j}")
                    nc.vector.reduce_sum(den[:], dslots[:, 0:ng],
                                         axis=mybir.AxisListType.X)
                    nc.vector.reciprocal(rcp[:], den[:])
                    Ph[j], rch[j] = P, rcp

                def ptr(j):
                    ext = _ext(j)
                    P = Ph[j]
                    PT = ptp.tile([128, NB * 128], bf16, tag="PT",
                                  name=f"PT{j}")
                    for g in range((ext + 3) // 4):
                        nb = min(4, ext - g * 4)
                        tps = ps_t.tile([128, 512], bf16, tag="tp",
                                        name=f"tps{j}_{g}")
                        for bb in range(nb):
                            nc.tensor.transpose(
                                tps[:, bb * 128:(bb + 1) * 128],
                                P[:, g * 512 + bb * 128:g * 512 + bb * 128 + 128],
                                ident[:])
                        nc.vector.tensor_copy(
                            PT[:, g * 512:g * 512 + nb * 128],
                            tps[:, 0:nb * 128])
                    return PT

                def ctx_mm(j, PT):
                    ext = _ext(j)
                    ctx = cp.tile([128, D], bf16, tag="ctx", name=f"ctx{j}")
                    for og in range(2):
                        ops = ps_m.tile([128, 512], f32, tag="av",
                                        name=f"av{j}_{og}")
                        for kb in range(ext):
                            nc.tensor.matmul(
                                ops[:],
                                PT[:, kb * 128:(kb + 1) * 128],
                                xnat[:, kb * D + og * 512:kb * D + og * 512 + 512],
                                start=(kb == 0), stop=(kb == ext - 1))
                        nc.vector.tensor_scalar_mul(
                            ctx[:, og * 512:(og + 1) * 512], ops[:],
                            rch[j][:])
                    cth[j] = ctx

                def ctxT_tr(j):
                    ctx = cth[j]
                    ctxT = ctp.tile([128, D], bf16, tag="ctxT",
                                    name=f"ctxT{j}")
                    for h in range(2):
                        tps = ps_t.tile([128, 512], bf16, tag="tp",
                                        name=f"tpc{j}_{h}")
                        for q4 in range(4):
                            dc = h * 4 + q4
                            nc.tensor.transpose(
                                tps[:, q4 * 128:(q4 + 1) * 128],
                                ctx[:, dc * 128:dc * 128 + 128],
                                ident[:])
                        nc.vector.tensor_copy(
                            ctxT[:, h * 512:(h + 1) * 512], tps[:])
                    return ctxT

                def out_mm(j, ctxT):
                    osb = op.tile([128, D], f32, tag="o", name=f"o{j}")
                    for og in range(2):
                        ops = ps_m.tile([128, 512], f32, tag="av",
                                        name=f"op{j}_{og}")
                        for dc in range(DC):
                            nc.tensor.matmul(
                                ops[:],
                                ctxT[:, dc * 128:dc * 128 + 128],
                                wv[:, dc * 1024 + og * 512:dc * 1024 + og * 512 + 512],
                                start=(dc == 0), stop=(dc == DC - 1))
                        nc.vector.tensor_copy(
                            osb[:, og * 512:(og + 1) * 512], ops[:])
                    nc.sync.dma_start(out_d[j * 128:(j + 1) * 128, :], osb[:])

                # software pipeline: S(0) Ptr(0) ctx(0) S(1) |
                #   j: ctxT(j-1) Ptr(j) out(j-1) ctx(j) S(j+1) | ctxT(7) out(7)
                scores_exp(0)
                PT = ptr(0)
                ctx_mm(0, PT)
                scores_exp(1)
                for j in range(1, NT):
                    cT = ctxT_tr(j - 1)
                    PT = ptr(j)
                    out_mm(j - 1, cT)
                    ctx_mm(j, PT)
                    if j + 1 < NT:
                        scores_exp(j + 1)
                cT = ctxT_tr(NT - 1)
                out_mm(NT - 1, cT)

    nc.compile()
    _cache["nc"] = nc
    return nc


def _shard(x, Wq, Wk, Wv):
    """Build the 8 per-core input maps, pre-arranged to SBUF layouts."""
    ident = np.eye(128, dtype=np.float32).astype(BF16)
    # wq2[p, ot*1024+dc*128+oo] = Wq[ot*128+oo, dc*128+p]
    wq2 = np.ascontiguousarray(
        Wq.reshape(8, 128, 8, 128).transpose(3, 0, 2, 1)
        .reshape(128, 8192)).astype(BF16)
    # wk2[p, dc*1024+oc*128+dd] = Wk[oc*128+p, dc*128+dd]
    wk2 = np.ascontiguousarray(
        Wk.reshape(8, 128, 8, 128).transpose(1, 2, 0, 3)
        .reshape(128, 8192)).astype(BF16)
    # wv2[p, dc*1024+o] = Wv[o, dc*128+p]
    wv2 = np.ascontiguousarray(
        Wv.T.reshape(8, 128, 1024).transpose(1, 0, 2)
        .reshape(128, 8192)).astype(BF16)
    in_maps = []
    for c in range(8):
        b, p = c // 2, c % 2
        xb = np.asarray(x[b])
        # xkv2[p, dc*2048+kv] = x[kv, dc*128+p]
        xkv2 = np.ascontiguousarray(
            xb.T.reshape(8, 128, 2048).transpose(1, 0, 2)
            .reshape(128, 16384)).astype(BF16)
        # xn2[p, kb*1024+d] = x[kb*128+p, d]
        xn2 = np.ascontiguousarray(
            xb.reshape(16, 128, 1024).transpose(1, 0, 2)
            .reshape(128, 16384)).astype(BF16)
        rows = np.concatenate(
            [xb[(2 * j + p) * 128:(2 * j + p + 1) * 128, :] for j in range(8)],
            axis=0)
        # xq2[p, sg*4096+dc*512+ss] = rows[sg*512+ss, dc*128+p]
        xq2 = np.ascontiguousarray(
            rows.reshape(2, 512, 8, 128).transpose(3, 0, 2, 1)
            .reshape(128, 8192)).astype(BF16)
        masks = np.full((NT * 128, 512), NEG, np.float32)
        for j in range(NT):
            ext = _ext(j)
            ng = (ext + 3) // 4
            gw = min(512, ext * 128 - (ng - 1) * 512)
            q_abs = (2 * j + p) * 128 + np.arange(128)[:, None]
            kv_abs = (ng - 1) * 512 + np.arange(gw)[None, :]
            masks[j * 128:(j + 1) * 128, 0:gw] = np.where(
                kv_abs <= q_abs, np.float32(0), NEG)
        masks2 = np.ascontiguousarray(
            masks.reshape(8, 128, 512).transpose(1, 0, 2)
            .reshape(128, 4096)).astype(BF16)
        in_maps.append({
            "xq": xq2, "xkv": xkv2, "xn": xn2,
            "wq": wq2, "wk": wk2, "wv": wv2,
            "masks": masks2, "ident": ident,
        })
    return in_maps


def _unshard(results, dtype):
    out = np.empty((B, S, D), dtype)
    for c in range(8):
        b, p = c // 2, c % 2
        o = results[c]["out"]
        for j in range(NT):
            out[b, (2 * j + p) * 128:(2 * j + p + 1) * 128, :] = \
                o[j * 128:(j + 1) * 128, :]
    return out


def run(x, Wq, Wk, Wv, trace=False):
    from concourse.bass_utils import run_bass_kernel_spmd
    nc = _build()
    in_maps = _shard(np.asarray(x), np.asarray(Wq), np.asarray(Wk),
                     np.asarray(Wv))
    res = run_bass_kernel_spmd(nc, in_maps, core_ids=list(range(8)),
                               trace=trace)
    return _unshard(res.results, np.float32), res


def kernel(x, Wq, Wk, Wv):
    out, _ = run(x, Wq, Wk, Wv, trace=False)
    return out
